# revision 72
# baseline (speedup 1.0000x reference)
"""CRF forward (-log-likelihood) Trainium2 kernel — meet-in-the-middle scan.

Math (per batch b, all-ones mask, L=512, C=128):
  scores_b = sum_t T[tags[t],tags[t+1]] + sum_{t=1..510} em[b,t,tags[t]]
  logZ_b   = forward-algorithm log partition
  out      = sum_b (logZ_b - scores_b)

The log-partition is a chain product evaluated from BOTH ends to halve the
serial depth (509 -> 255 dependent matmul+mult steps):

  logZ = ln( v^T A_510 ),  v = exp(T[:,1])
  A_510 = S_510 ... S_2 A_1,     S_t x = F_t (.) (ETc^T x),  F_t = exp(e_t)
  split at MID:  logZ = ln( w_{MID+1}^T A_MID )
    fwd:  A_t   = F_t (.) (ETc^T A_{t-1}),    t = 2..MID
    bwd:  w_t   = ETc (F_t (.) w_{t+1}),      t = 510..MID+1
  ETc = exp(T - CDRIFT); the 509 drift factors are added back on host.

Each direction is split into batch-column chains; every chain's per-step
recurrence is one PE matmul (PSUM) + one elementwise multiply (tensor_tensor)
on the engine given by the chain map.  exp(e) ("F") tiles are produced off
the critical path: HBM load (f32) -> ACT exp (bf16) -> DMA xbar transpose to
[C, cols] layout; loads round-robin over the SP/ACT/(DVE) rings.

Gold-path scores are computed on host (float64 gathers).

Sharding: batch 512 -> 8 cores x 64 (SPMD, same NEFF, different shards).
"""

import os
import numpy as np
from contextlib import ExitStack

import concourse.bass as bass
import concourse.tile as tile
from concourse import bacc, mybir
from concourse import bass_utils

B, L, C = 512, 512, 128
NCORES = 8
BLOC = B // NCORES  # 64
CDRIFT = 5.33
CS = 17             # s-slots per chunk (covers 34 time steps)
NSLOT = 255         # (s, par) slots: t = 1 + 2*s + par, t in [1, 510]
NCHUNK = NSLOT // CS  # 15
MID = 255           # fwd covers t=2..MID, bwd covers t=MID+1..510
NF = MID - 1        # 254 fwd rounds
NB = L - 2 - MID    # 255 bwd matmul rounds (t=510..256)
MINI = int(os.environ.get("KERN_MINI", "8"))  # prologue slots per end

DVE_COLS = int(os.environ.get("KERN_DVECOLS", "0"))
NSPLIT = int(os.environ.get("KERN_NSPLIT", "1"))
B14R = os.environ.get("KERN_B14R", "ss")   # rings for bwd end-chunk pars
P1T = os.environ.get("KERN_P1T", "sy_sc")  # transpose rings for pair 1
DRAIN = int(os.environ.get("KERN_DRAIN", "4"))

F32 = mybir.dt.float32
BF16 = mybir.dt.bfloat16
AF = mybir.ActivationFunctionType
ALU = mybir.AluOpType


def build_kernel():
    nc = bacc.Bacc("TRN2", target_bir_lowering=False, debug=False,
                   enable_asserts=False, num_devices=NCORES)

    em_d = nc.dram_tensor("em", [BLOC, L, C], F32, kind="ExternalInput").ap()
    tr_d = nc.dram_tensor("tr", [C, C], F32, kind="ExternalInput").ap()
    out_d = nc.dram_tensor("partial", [1, 1], F32, kind="ExternalOutput").ap()

    with tile.TileContext(nc) as tc, ExitStack() as ctx:
        const_p = ctx.enter_context(tc.tile_pool(name="const", bufs=1))
        echunk_p = ctx.enter_context(tc.tile_pool(name="echunk", bufs=6))
        fconv_p = ctx.enter_context(tc.tile_pool(name="fconv", bufs=11))
        ft_p = ctx.enter_context(tc.tile_pool(name="ft", bufs=NCHUNK + 2))
        a_p = ctx.enter_context(tc.tile_pool(name="astate", bufs=3))
        ps_p = ctx.enter_context(tc.tile_pool(name="ps", bufs=1, space="PSUM"))
        psf_p = ctx.enter_context(tc.tile_pool(name="psf", bufs=1, space="PSUM"))
        tp_p = ctx.enter_context(tc.tile_pool(name="tp", bufs=3, space="PSUM"))
        fin_p = ctx.enter_context(tc.tile_pool(name="fin", bufs=1))

        # ---- F pipeline: load -> exp -> transpose ----
        # e sub-chunk layout: partition p = par*64 + b ; free = (s_loc, c)
        # holds t = 1 + 2*s + par for s in [s0, s1)
        # Three-stage pipeline, each stage one pair further ahead so no
        # stage ever waits a cross-ring dep at its queue head:
        #   DMAs (2/3 SP, 1/3 ACT)          - 3 pairs ahead
        #   exp (ACT)                       - 2 pairs ahead
        #   transposes                      - 1 pair ahead
        # Steady-state transposes run on the PE (transpose-matmul into a
        # rolling PSUM buffer) with DVE draining PSUM->SBUF; this keeps the
        # 8 shared HWDGE semaphore lanes free for the actual HBM loads.
        # Prologue chunks use HWDGE DmaTranspose (PE/DVE path needs the
        # round loop to drip-feed it).
        emr = em_d[:, 1:L - 1, :].rearrange("b (s par) c -> par b s c", par=2)
        slot_map = {}   # s -> (ft_tile, local_slot)
        dma_cnt = [0]
        tqf = []  # pending fwd transpose work: (ft, fc, sl)
        tqb = []  # pending bwd transpose work

        def emit_dma(s0, s1, desc, ring=None):
            """Issue the HBM load for slots [s0, s1); returns finish token."""
            n = s1 - s0
            ec = echunk_p.tile([C, n, C], F32, tag="ec", name="ec")
            for par in (0, 1):
                if isinstance(ring, tuple):
                    r = ring[par]
                elif ring is not None:
                    r = ring
                else:
                    r = nc.scalar if dma_cnt[0] % 3 == 2 else nc.sync
                    dma_cnt[0] += 1
                r.dma_start(ec[par * BLOC:(par + 1) * BLOC, :, :],
                            emr[par, :, s0:s1, :])
            return [s0, s1, desc, ec, None]

        def emit_exp(tok):
            s0, s1, desc, ec, _ = tok
            n = s1 - s0
            fc = fconv_p.tile([C, n, C], BF16, tag="fc", name="fc")
            halves = [(0, (n + 1) // 2), ((n + 1) // 2, n)]
            if desc:
                halves = halves[::-1]
            for h0, h1 in halves:
                if h1 > h0:
                    nc.scalar.activation(fc[:, h0:h1, :], ec[:, h0:h1, :],
                                         AF.Exp)
            tok[4] = fc

        tcnt = [0]

        def emit_T(tok, eager=False):
            s0, s1, desc, ec, fc = tok
            n = s1 - s0
            # small prologue tiles get their own tag so the big-chunk tag's
            # slot size (and SBUF footprint) isn't applied to them
            tg, nb = ("ft", 13) if n >= CS else ("fts", 4)
            ft = ft_p.tile([C, n, C], BF16, tag=tg, name="ft", bufs=nb)
            order = range(n - 1, -1, -1) if desc else range(n)
            q = tqb if desc else tqf
            for sl in order:
                slot_map[s0 + sl] = (ft, sl)
                if eager:
                    # prologue: PE transpose + DVE copy run immediately (both
                    # engines idle before the scan), skipping the ring queues
                    # already loaded with bulk pair DMAs
                    tps = tp_p.tile([C, C], BF16, tag="tp", name="tps")
                    nc.tensor.transpose(tps[:], fc[:, sl, :], ident[:])
                    nc.vector.tensor_copy(ft[:, sl, :], tps[:])
                else:
                    q.append((ft, fc, sl))

        def drain_T(nmax):
            # alternate fwd/bwd so both chains stay fed; transposes ride
            # the HWDGE rings, alternating to balance
            for j in range(nmax):
                q = (tqf, tqb)[j % 2] or (tqb, tqf)[j % 2]
                if not q:
                    break
                ft, fc, sl = q.pop(0)
                eng = (nc.sync, nc.scalar)[tcnt[0] % 2]
                tcnt[0] += 1
                eng.dma_start(ft[:, sl, :], fc[:, sl, :], transpose=True)

        def emit_finish(tok, eager=False):
            emit_exp(tok)
            emit_T(tok, eager)

        def fslice(t, c0, c1):
            lin = t - 1
            s, par = lin // 2, lin % 2
            ft, sl = slot_map[s]
            return ft[:, sl, par * BLOC + c0:par * BLOC + c1]

        # prologue: tiny end slices (loaded on the gpsimd ring) so both
        # chains start fast.  Constants queue behind the minis.
        tk_m0 = emit_dma(0, MINI, False, ring=nc.gpsimd)
        tk_m14 = emit_dma(NSLOT - MINI, NSLOT, True, ring=nc.gpsimd)

        # constants / transition-derived tiles (gpsimd + ACT rings)
        t_sb = const_p.tile([C, C], F32)
        nc.gpsimd.dma_start(t_sb[:], tr_d[:])
        tendcol = const_p.tile([C, 1], F32)  # T[:, 1] column
        nc.gpsimd.dma_start(tendcol[:], tr_d[:, 1:2])
        trT = const_p.tile([C, C], F32)
        nc.scalar.dma_start(trT[:], tr_d.rearrange("a b -> b a"))
        t0col = const_p.tile([C, 1], F32)  # T[0, :] as a column
        nc.scalar.dma_start(t0col[:], tr_d[0:1, :].rearrange("a b -> b a"))
        negc = const_p.tile([C, 1], F32)
        nc.vector.memset(negc[:], -CDRIFT)
        ones = const_p.tile([C, 1], F32)
        nc.vector.memset(ones[:], 1.0)

        # bf16 identity for the prologue PE transposes (SBUF-only gpsimd ops)
        ident = const_p.tile([C, C], BF16)
        nc.vector.memset(ident[:], 0.0)
        nc.gpsimd.iota(ident[:].bitcast(mybir.dt.uint16),
                       pattern=[[0, C]], base=0x3f80,
                       channel_multiplier=0)
        nc.gpsimd.affine_select(ident[:], ident[:], pattern=[[-1, C]],
                                compare_op=ALU.is_equal, fill=0.0,
                                base=0, channel_multiplier=1)

        # mini exps ahead of the constant exps on ACT
        emit_finish(tk_m0, eager=True)
        emit_finish(tk_m14, eager=True)

        etc = const_p.tile([C, C], BF16)
        nc.scalar.activation(etc[:], t_sb[:], AF.Exp, bias=negc[:])
        etcT = const_p.tile([C, C], BF16)
        nc.scalar.activation(etcT[:], trT[:], AF.Exp, bias=negc[:])
        expt0 = const_p.tile([C, 1], F32)
        nc.scalar.activation(expt0[:], t0col[:], AF.Exp)
        exptend = const_p.tile([C, 1], F32)
        nc.scalar.activation(exptend[:], tendcol[:], AF.Exp)

        dma_q = []   # tokens DMA-issued, exp not yet emitted
        exp_q = []   # tokens exp-emitted, transpose not yet emitted

        # ---- chains ----
        # scan multiplies must run on DVE: it is the only vector engine
        # with PSUM access (GPSIMD/Pool cannot touch PSUM on real HW)
        w = BLOC // NSPLIT
        chains = [{"eng": nc.vector, "c0": g * w, "c1": (g + 1) * w,
                   "i": g, "w": w} for g in range(NSPLIT)]

        # stream remaining chunk pairs; pair p -> chunks (p, 14-p)
        def emit_pair_dma(p):
            if p <= 6:
                dma_q.append([emit_dma(CS * p, CS * (p + 1), False),
                              emit_dma(CS * (14 - p), CS * (15 - p), True)])
            elif p == 7:
                dma_q.append([emit_dma(CS * 7, CS * 8, False)])

        def advance_pipeline():
            if exp_q:
                for tok in exp_q.pop(0):
                    emit_T(tok)
            if dma_q:
                toks = dma_q.pop(0)
                for tok in toks:
                    emit_exp(tok)
                exp_q.append(toks)

        # prologue: rest of the end chunks, then pair 1; all transposes go
        # through the PE/DVE queues (drained in the round loop)
        tk_b0 = emit_dma(MINI, CS, False, ring=(nc.sync, nc.scalar))
        tk_b14 = emit_dma(CS * (NCHUNK - 1), NSLOT - MINI, True,
                          ring=(nc.scalar, nc.sync))
        emit_pair_dma(1)
        emit_finish(tk_b0, eager=True)
        emit_finish(tk_b14, eager=True)
        emit_pair_dma(2)
        for tok in dma_q.pop(0):  # pair 1
            emit_finish(tok, eager=True)
        advance_pipeline()  # exp pair 2
        emit_pair_dma(3)

        for k in range(max(NF, NB)):
            nq = len(tqf) + len(tqb)
            drain_T(DRAIN if nq > 20 else (2 if nq > 8 else 1))
            if k == 0:
                # inits: A_1 = F_1 (.) exp(T[0,:]); x_510 = F_510 (.) exp(T[:,1])
                for ch in chains:
                    i, c0, c1, w = ch["i"], ch["c0"], ch["c1"], ch["w"]
                    a = a_p.tile([C, w], BF16, tag=f"af{i}", name=f"af{i}")
                    ch["eng"].tensor_scalar_mul(a[:], fslice(1, c0, c1),
                                                expt0[:])
                    ch["af"] = a
                    x = a_p.tile([C, w], BF16, tag=f"ax{i}", name=f"ax{i}")
                    ch["eng"].tensor_scalar_mul(x[:], fslice(510, c0, c1),
                                                exptend[:])
                    ch["ax"] = x
            if k > 0 and k % 34 == 0:
                advance_pipeline()
                emit_pair_dma(k // 34 + 3)
            # bwd matmul: w_{510-k} = ETc @ x_{510-k}
            if k < NB:
                for ch in chains:
                    i, w = ch["i"], ch["w"]
                    ps = ps_p.tile([C, w], F32, tag=f"pb{i}", name=f"pb{i}")
                    nc.tensor.matmul(out=ps[:], lhsT=etcT[:], rhs=ch["ax"][:],
                                     start=True, stop=True)
                    ch["psb"] = ps
            # fwd matmul: u = ETc^T @ A_{1+k}
            if k < NF:
                for ch in chains:
                    i, w = ch["i"], ch["w"]
                    ps = ps_p.tile([C, w], F32, tag=f"pf{i}", name=f"pf{i}")
                    nc.tensor.matmul(out=ps[:], lhsT=etc[:], rhs=ch["af"][:],
                                     start=True, stop=True)
                    ch["psf"] = ps
            # bwd mult: x_{509-k} = F_{509-k} (.) w_{510-k}
            if k < NB - 1:
                for ch in chains:
                    i, c0, c1, w = ch["i"], ch["c0"], ch["c1"], ch["w"]
                    x = a_p.tile([C, w], BF16, tag=f"ax{i}", name=f"ax{i}")
                    ch["eng"].tensor_tensor(out=x[:], in0=ch["psb"][:],
                                            in1=fslice(509 - k, c0, c1),
                                            op=ALU.mult)
                    ch["ax"] = x
            # fwd mult: A_{2+k} = u (.) F_{2+k}
            if k < NF:
                for ch in chains:
                    i, c0, c1, w = ch["i"], ch["c0"], ch["c1"], ch["w"]
                    a = a_p.tile([C, w], BF16, tag=f"af{i}", name=f"af{i}")
                    ch["eng"].tensor_tensor(out=a[:], in0=ch["psf"][:],
                                            in1=fslice(2 + k, c0, c1),
                                            op=ALU.mult)
                    ch["af"] = a

        # ---- finalization: logZ_b = ln( sum_j w_256[j,b] * A_255[j,b] ) ----
        fin = fin_p.tile([C, BLOC], F32)
        for ch in chains:
            c0, c1 = ch["c0"], ch["c1"]
            ch["eng"].tensor_tensor(out=fin[:, c0:c1], in0=ch["psb"][:],
                                    in1=ch["af"][:], op=ALU.mult)
        fps = psf_p.tile([1, BLOC], F32)
        nc.tensor.matmul(out=fps[:], lhsT=ones[:], rhs=fin[:], start=True,
                         stop=True)
        lnv = fin_p.tile([1, BLOC], F32)
        nc.scalar.activation(lnv[:], fps[:], AF.Ln)
        part = fin_p.tile([1, 1], F32)
        nc.vector.tensor_reduce(part[:], lnv[:], axis=mybir.AxisListType.X,
                                op=ALU.add)
        nc.sync.dma_start(out_d[:], part[:])

    nc.compile()
    return nc


_NC_CACHE = None


def _get_nc():
    global _NC_CACHE
    if _NC_CACHE is None:
        _NC_CACHE = build_kernel()
    return _NC_CACHE


def kernel(emissions, tags, mask, transitions):
    emissions = np.ascontiguousarray(np.asarray(emissions, dtype=np.float32))
    tags = np.asarray(tags).astype(np.int32)
    mask = np.asarray(mask, dtype=np.float32)
    transitions = np.ascontiguousarray(
        np.asarray(transitions, dtype=np.float32))
    assert emissions.shape == (B, L, C) and tags.shape == (B, L)
    assert np.all(mask == 1.0), "kernel assumes an all-ones mask"

    # gold-path scores on host (the HW indirect-DMA path only supports
    # per-partition run gathers, not per-element gathers)
    T64 = transitions.astype(np.float64)
    t_score = T64[tags[:, :L - 1], tags[:, 1:]].sum(1)
    e_score = np.take_along_axis(
        emissions.astype(np.float64), tags[..., None], 2)[..., 0][:, 1:L - 1].sum(1)
    scores_total = float((t_score + e_score).sum())

    nc = _get_nc()
    in_maps = [{"em": emissions[cid * BLOC:(cid + 1) * BLOC],
                "tr": transitions} for cid in range(NCORES)]
    res = bass_utils.run_bass_kernel_spmd(nc, in_maps,
                                          core_ids=list(range(NCORES)))
    total = sum(float(r["partial"][0, 0]) for r in res.results)
    total += B * (L - 3) * CDRIFT - scores_total
    return np.float32(total)
